# revision 48
# baseline (speedup 1.0000x reference)
"""Trainium2 Bass kernel for a GRU-like recurrent cell (4 unrolled timesteps)
with relu candidate and final output projection.

Math (per batch row, h0 = 0):
  for t in 0..3:
    r = sigmoid(x_t @ wr + h @ Ur + br)        # skipped at t=0 (r*h = 0)
    z = sigmoid(x_t @ wz + h @ Uz + bz)
    c = relu  (x_t @ wh + (r*h) @ Uh + bh)
    h = (1-z)*c + z*h
  y = relu(h @ w_out + b_out)

Distribution: data-parallel over batch across 8 cores (x/y sharded on dim 0,
weights replicated). Each core computes B_LOC=1024 rows.

fp8 (e4m3) DoubleRow matmuls (2 contraction elems/partition/cycle) for the
error-tolerant sites (x@wr, x@wz, h@Ur, h@Uz, (r*h)@Uh); bf16 for the
error-critical sites (x@wh, h@w_out).  Scales: weights x256, U matrices x32,
h-state x8 -> every gate PSUM holds 256*(true preactivation), dequantized for
free by ACT (out = func(psum*(1/256) + bias)).  wh is scaled x256 in bf16 so
the mixed bf16+fp8 accumulation shares one PSUM scale.

z is stored as wbar = 1-z = sigmoid(-pre) in bf16: saturated gates (z ~ 1,
driven by the positive-mean h @ Uz sum) need relative precision on 1-z.
h update: h' = h - wbar*(h - hc); t=0: h1 = wbar*hc (on the otherwise-idle
GPSIMD engine, so the one-time DVE weight-cast chain cannot stall it).

All recurrent state is kept TRANSPOSED in SBUF as [h_partition, batch_free]
tiles.  x is cast fp32->bf16 into a DRAM scratch (ACT) then loaded transposed
via the 2-byte xbar DMA transpose (sync queue only - xbar descriptors are
broken on the ACT HWDGE queue); fp8 copies of the transposed tiles are made
by ACT casts in SBUF.

Weights are loaded fp32 once and cast+scaled on-chip (DVE): wr/wz into packed
fp8 pair-tile DRAM staging ([128, 2, 1024] fp8: sub-tile i = contraction rows
128i..128i+127 of a 256-row pair block), U matrices likewise ([128, 2, 1024]
with 128-row sub-blocks), wh into per-output-block packed bf16 staging
([ht][kd][128][128]) so the candidate stage keeps only 3 of 8 column windows
in SBUF.  Everything is re-streamed per step from staging on the sync queue
in consumption order; all one-time fp32 loads ride the sync queue in deadline
order (Q7/SWDGE measured ~44 GB/s - too slow for anything deadline-bound).
"""
import numpy as np

B_FULL, T, D, H, U = 8192, 4, 2048, 1024, 2048
N_CORES = 8
B_LOC = B_FULL // N_CORES   # 1024
BC = 512                    # batch columns per moving-operand chunk
NBC = B_LOC // BC           # 2
KD = D // 128               # 16 contraction tiles for x @ W
KDP = KD // 2               # 8 fp8 pair tiles
KH = H // 128               # 8 contraction tiles for h @ U
KHP = KH // 2               # 4 fp8 pair tiles
NUC = U // BC               # 4 output column chunks
NBI = BC // 128             # 4 output row tiles per chunk

SW = 256.0                  # weight scale (wr, wz, wh)
SU = 32.0                   # U matrix scale
SH = 8.0                    # h state scale  (SW = SU * SH)

S32_BUFS = 2
XS16_BUFS = 2
XT16_BUFS = 49     # 32 hard-live in c stage + 16 so next step's bc0
                   # transposes can run during c + 1
XT8_BUFS = 16      # 16 hard-live per step (8 kdp x 2 bc)
W8_BUFS = 15
WHP_BUFS = 5
WH16N_BUFS = 3
H_BUFS = 18
H8_BUFS = 9
RH8_BUFS = 8       # all 8 (4 khp x 2 bc) live through c stage
WBAR_BUFS = 17     # all 16 (8 ht x 2 bc) live into c stage + 1
R_BUFS = 3
HC_BUFS = 2
DE_BUFS = 2


def _build():
    import concourse.mybir as mybir
    import concourse.tile as tile
    import concourse.bass as bass
    from concourse import bacc

    f32 = mybir.dt.float32
    bf16 = mybir.dt.bfloat16
    fp8 = mybir.dt.float8e4
    Act = mybir.ActivationFunctionType
    DR = mybir.MatmulPerfMode.DoubleRow

    def sl(i, step=128):
        return slice(i * step, (i + 1) * step)

    nc = bacc.Bacc("TRN2", target_bir_lowering=False, name="gru_fp8")

    x_in = nc.dram_tensor("x", [B_LOC, T, D], f32, kind="ExternalInput")
    w_in = {
        "r": nc.dram_tensor("wr", [D, H], f32, kind="ExternalInput"),
        "z": nc.dram_tensor("wz", [D, H], f32, kind="ExternalInput"),
        "c": nc.dram_tensor("wh", [D, H], f32, kind="ExternalInput"),
    }
    u_in = {
        "r": nc.dram_tensor("Ur", [H, H], f32, kind="ExternalInput"),
        "z": nc.dram_tensor("Uz", [H, H], f32, kind="ExternalInput"),
        "c": nc.dram_tensor("Uh", [H, H], f32, kind="ExternalInput"),
    }
    b_in = {
        "r": nc.dram_tensor("br", [H], f32, kind="ExternalInput"),
        "z": nc.dram_tensor("bz", [H], f32, kind="ExternalInput"),
        "c": nc.dram_tensor("bh", [H], f32, kind="ExternalInput"),
    }
    wout_in = nc.dram_tensor("w_out", [H, U], f32, kind="ExternalInput")
    bout_in = nc.dram_tensor("b_out", [U], f32, kind="ExternalInput")
    y_out = nc.dram_tensor("y", [B_LOC, U], f32, kind="ExternalOutput")
    xbf = nc.dram_tensor("xbf", [T, B_LOC, D], bf16)
    # packed fp8 pair-tile staging: [kdp][part 128][sub 2][col 1024]
    w8s = {g: nc.dram_tensor(f"w8s_{g}", [KDP, 128, 2, H], fp8)
           for g in ("r", "z")}
    u8s = {g: nc.dram_tensor(f"u8s_{g}", [KHP, 128, 2, H], fp8)
           for g in ("r", "z", "c")}
    # wh packed per ht: [ht 8][kd 16][part 128][col 128] bf16 (x256)
    whp_s = nc.dram_tensor("whp_s", [KH, KD, 128, 128], bf16)

    with tile.TileContext(nc) as tc:
        with tc.tile_pool(name="sb", bufs=1) as sb, \
             tc.tile_pool(name="ps", bufs=6, space="PSUM") as ps:

            # ---- biases: [128, KH] per-partition scalars per h-tile ----
            bias_sb = {}
            for g in ("r", "z", "c"):
                bt = sb.tile([128, KH], f32, name=f"bias_{g}", tag=f"bias_{g}")
                nc.sync.dma_start(bt, b_in[g].ap().rearrange("(kh p) -> p kh", p=128))
                bias_sb[g] = bt
            # negated bz for wbar = sigmoid(-pre - bz)
            bzn_sb = sb.tile([128, KH], f32, name="bzn", tag="bzn")
            nc.vector.tensor_scalar_mul(bzn_sb, bias_sb["z"], -1.0)
            # output bias broadcast across partitions: [128, U]
            bout_ap = bout_in.ap()
            bout_bcast_src = bass.AP(
                tensor=bout_ap.tensor, offset=bout_ap.offset,
                ap=[[0, 128]] + list(bout_ap.ap))
            bout_sb = sb.tile([128, U], bf16, name="bout_sb", tag="bout_sb")
            nc.gpsimd.dma_start(bout_sb, bout_bcast_src)

            # ---- x pipeline: fp32 -> bf16 xbf scratch, then xbar loads ----
            xts_all = {}   # (t) -> {(bc, kd): bf16 [128, 512] tile}
            xt8_all = {}   # (t) -> {(bc, kdp): fp8 [128, 2, 512] tile}

            def emit_x_pipe_quad(t, bc, half):
                for blk in range(4):
                    b0 = bc * BC + blk * 128
                    xs32 = sb.tile([128, 1024], f32,
                                   name=f"xs32_t{t}b{bc}h{half}k{blk}",
                                   tag="s32", bufs=S32_BUFS)
                    nc.sync.dma_start(
                        xs32, x_in[b0:b0 + 128, t, sl(half, 1024)])
                    xs16 = sb.tile([128, 1024], bf16,
                                   name=f"xs16_t{t}b{bc}h{half}k{blk}",
                                   tag="xs16", bufs=XS16_BUFS)
                    nc.scalar.copy(xs16, xs32)
                    nc.sync.dma_start(
                        xbf[t, b0:b0 + 128, sl(half, 1024)], xs16)

            def emit_xbars_kds(t, bc, kds):
                xts = xts_all.setdefault(t, {})
                for kd in kds:
                    xt_t = sb.tile([128, BC], bf16,
                                   name=f"xt_t{t}b{bc}k{kd}", tag="xt",
                                   bufs=XT16_BUFS)
                    nc.sync.dma_start(
                        xt_t, xbf[t, sl(bc, BC), sl(kd)], transpose=True)
                    xts[(bc, kd)] = xt_t

            def emit_xbars_quad(t, bc, half):
                emit_xbars_kds(t, bc, range(half * 8, half * 8 + 8))

            def emit_xt8_bc(t, bc, kdps=None):
                # ACT casts bf16 -> fp8 pair tiles [128, 2, 512]
                xts = xts_all[t]
                x8 = xt8_all.setdefault(t, {})
                for kdp in kdps if kdps is not None else range(KDP):
                    t8 = sb.tile([128, 2, BC], fp8,
                                 name=f"xt8_t{t}b{bc}p{kdp}", tag="xt8",
                                 bufs=XT8_BUFS)
                    for i in range(2):
                        nc.scalar.copy(t8[:, i, :], xts[(bc, 2 * kdp + i)])
                    x8[(bc, kdp)] = t8

            def emit_xt8(t):
                for bc in range(NBC):
                    emit_xt8_bc(t, bc)

            # ---- one-time weight cast + pack-store pipelines (all sync) ----
            def emit_w8_cast(g, pair_range=None, retain=False):
                # w[g] fp32 [D, H] -> pair tiles [128, 2, H] fp8 x SW
                tiles = []
                for kdp in pair_range if pair_range is not None else range(KDP):
                    w8t = sb.tile([128, 2, H], fp8, name=f"w8c_{g}{kdp}",
                                  tag="w8", bufs=W8_BUFS)
                    for i in range(2):
                        s32 = sb.tile([128, 1024], f32,
                                      name=f"wc32_{g}{kdp}i{i}",
                                      tag="s32", bufs=S32_BUFS)
                        nc.sync.dma_start(s32, w_in[g][sl(2 * kdp + i), :])
                        nc.vector.tensor_scalar_mul(w8t[:, i, :], s32, SW)
                    nc.sync.dma_start(w8s[g][kdp], w8t)
                    if retain:
                        tiles.append(w8t)
                return tiles

            def emit_u8_cast(g, ld=None):
                ld = ld or nc.sync
                for khp in range(KHP):
                    u8t = sb.tile([128, 2, H], fp8, name=f"u8c_{g}{khp}",
                                  tag="w8", bufs=W8_BUFS)
                    for i in range(2):
                        s32 = sb.tile([128, 1024], f32,
                                      name=f"uc32_{g}{khp}i{i}",
                                      tag="s32", bufs=S32_BUFS)
                        ld.dma_start(s32, u_in[g][sl(2 * khp + i), :])
                        nc.vector.tensor_scalar_mul(u8t[:, i, :], s32, SU)
                    nc.sync.dma_start(u8s[g][khp], u8t)

            def emit_wh_cast():
                # wh fp32 [D, H] -> x256 bf16, packed [ht][kd][128][128]
                whp_ap = whp_s.ap()
                for kd in range(KD):
                    s32 = sb.tile([128, 1024], f32, name=f"whc32_{kd}",
                                  tag="s32", bufs=S32_BUFS)
                    nc.sync.dma_start(s32, w_in["c"][sl(kd), :])
                    w16 = sb.tile([128, KH, 128], bf16, name=f"wh16n_{kd}",
                                  tag="wh16n", bufs=WH16N_BUFS)
                    nc.vector.tensor_scalar_mul(w16, s32, SW)
                    # store (p, ht, m) -> whp_s[ht, kd, p, m]
                    dst = bass.AP(
                        tensor=whp_ap.tensor,
                        offset=whp_ap.offset + kd * 128 * 128,
                        ap=[[128, 128], [KD * 128 * 128, KH], [1, 128]])
                    nc.sync.dma_start(dst, w16)

            # ---- streamed loads (sync queue) ----
            def load_w8(g, t):
                tiles = []
                for kdp in range(KDP):
                    w8t = sb.tile([128, 2, H], fp8, name=f"w8_{g}{kdp}_t{t}",
                                  tag="w8", bufs=W8_BUFS)
                    nc.sync.dma_start(w8t, w8s[g][kdp])
                    tiles.append(w8t)
                return tiles

            def load_u8(g, t):
                tiles = []
                for khp in range(KHP):
                    u8t = sb.tile([128, 2, H], fp8, name=f"u8_{g}{khp}_t{t}",
                                  tag="w8", bufs=W8_BUFS)
                    nc.sync.dma_start(u8t, u8s[g][khp])
                    tiles.append(u8t)
                return tiles

            def load_whp(ht, t):
                # 2 tiles of [128, 8, 128] covering kd 0-7 / 8-15 for one ht
                whp_ap = whp_s.ap()
                tiles = []
                for j in range(2):
                    wt = sb.tile([128, 8, 128], bf16,
                                 name=f"whp_t{t}h{ht}j{j}", tag="whp",
                                 bufs=WHP_BUFS)
                    src = bass.AP(
                        tensor=whp_ap.tensor,
                        offset=whp_ap.offset
                        + ht * KD * 128 * 128 + j * 8 * 128 * 128,
                        ap=[[128, 128], [128 * 128, 8], [1, 128]])
                    nc.sync.dma_start(wt, src)
                    tiles.append(wt)
                return tiles

            # ---- recurrent state ----
            h = {}      # (kh, bc) -> bf16 [128, BC] hidden state (transposed)
            h8 = {}     # (khp, bc) -> fp8 [128, 2, BC], value = 8*h

            # t0 prologue on sync, deadline-ordered: x pipe + xbars + wz
            # interleaved (all gate the first MMs), then wh (t0-c via the
            # whp round trip), whp windows 0-2, then wr/Ur (t1-r), Uz/Uh.
            wz8_t0 = []
            for bc in range(NBC):
                for half in range(2):
                    emit_x_pipe_quad(0, bc, half)
                    emit_xbars_quad(0, bc, half)
                    i2 = 2 * (2 * bc + half)
                    wz8_t0 += emit_w8_cast("z", pair_range=range(i2, i2 + 2),
                                           retain=True)
            emit_xt8(0)
            emit_wh_cast()
            whp_t0 = {k: load_whp(k, 0) for k in range(3)}
            emit_w8_cast("r")
            emit_u8_cast("r")
            # Uz/Uh fp32 ride the ACT HWDGE queue (plain loads only - xbar
            # transposes are broken there), issued after the xt8(0) casts
            emit_u8_cast("z", ld=nc.scalar)
            emit_u8_cast("c", ld=nc.scalar)

            for t in range(T):
                xts = xts_all[t]
                x8 = xt8_all[t]
                rh8 = {}
                wbar = {}

                # ---------- r stage (t >= 1) ----------
                if t > 0:
                    w8r, u8r = w8r_pre, u8r_pre   # loaded at prior step tail
                    for bc in range(NBC):
                        for ht in range(KH):
                            p = ps.tile([128, BC], f32,
                                        name=f"pr_t{t}b{bc}h{ht}", tag="ps")
                            n = KDP + KHP
                            i = 0
                            # h@U first: x-independent work covers the
                            # step-boundary xbar/cast latency
                            for khp in range(KHP):
                                nc.tensor.matmul(
                                    p, u8r[khp][:, :, sl(ht)], h8[(khp, bc)],
                                    start=(i == 0), stop=False,
                                    perf_mode=DR)
                                i += 1
                            for kdp in range(KDP):
                                nc.tensor.matmul(
                                    p, w8r[kdp][:, :, sl(ht)], x8[(bc, kdp)],
                                    start=False, stop=(i == n - 1),
                                    perf_mode=DR)
                                i += 1
                            rt = sb.tile([128, BC], f32,
                                         name=f"r_t{t}b{bc}h{ht}",
                                         tag="r", bufs=R_BUFS)
                            nc.scalar.activation(
                                rt, p, Act.Sigmoid,
                                bias=bias_sb["r"][:, ht:ht + 1],
                                scale=1.0 / SW)
                            # rh8 pair tile: alloc at even ht, fill halves
                            khp_i, i_h = ht // 2, ht % 2
                            if i_h == 0:
                                rh8[(khp_i, bc)] = sb.tile(
                                    [128, 2, BC], fp8,
                                    name=f"rh8_t{t}b{bc}p{khp_i}",
                                    tag="rh8", bufs=RH8_BUFS)
                            nc.vector.tensor_mul(
                                rh8[(khp_i, bc)][:, i_h, :], rt,
                                h8[(ht // 2, bc)][:, ht % 2, :])

                # ---------- z stage ----------
                if t == 0:
                    w8z, u8z = wz8_t0, None
                else:
                    w8z = load_w8("z", t)
                    u8z = load_u8("z", t)
                for bc in range(NBC):
                    for ht in range(KH):
                        p = ps.tile([128, BC], f32,
                                    name=f"pz_t{t}b{bc}h{ht}", tag="ps")
                        n = KDP + (KHP if t > 0 else 0)
                        i = 0
                        if t > 0:
                            for khp in range(KHP):
                                nc.tensor.matmul(
                                    p, u8z[khp][:, :, sl(ht)], h8[(khp, bc)],
                                    start=(i == 0), stop=False,
                                    perf_mode=DR)
                                i += 1
                        for kdp in range(KDP):
                            nc.tensor.matmul(
                                p, w8z[kdp][:, :, sl(ht)], x8[(bc, kdp)],
                                start=(i == 0), stop=(i == n - 1),
                                perf_mode=DR)
                            i += 1
                        wb = sb.tile([128, BC], bf16,
                                     name=f"wbar_t{t}b{bc}h{ht}",
                                     tag="wbar", bufs=WBAR_BUFS)
                        nc.scalar.activation(
                            wb, p, Act.Sigmoid,
                            bias=bzn_sb[:, ht:ht + 1], scale=-1.0 / SW)
                        wbar[(ht, bc)] = wb

                # ---------- c stage + h update (ht-outer for whp windows) ---
                u8c = load_u8("c", t) if t > 0 else None
                h_new = {}
                h8_new = {}
                if t == 0:
                    whp_tiles = whp_t0
                else:
                    whp_tiles = {0: load_whp(0, t), 1: load_whp(1, t),
                                 2: load_whp(2, t)}
                for ht in range(KH):
                    if ht + 3 < KH:
                        whp_tiles[ht + 3] = load_whp(ht + 3, t)
                    if t < T - 1:
                        # next step's x pipe + bc0 transposes interleaved
                        # under c compute (xt slots 33-48 are free); each
                        # xbar trails its source quad by >= 2 iterations
                        if ht % 2 == 0:
                            emit_x_pipe_quad(t + 1, ht // 4, (ht // 2) % 2)
                        if 2 <= ht < 6:
                            emit_xbars_kds(t + 1, 0,
                                           range(4 * (ht - 2), 4 * ht - 4))
                        elif ht >= 6:
                            # bc0 fp8 casts can start once kd pairs exist
                            emit_xt8_bc(t + 1, 0, kdps=range(
                                4 * (ht - 6), 4 * ht - 20))
                    wja, wjb = whp_tiles[ht]
                    for bc in range(NBC):
                        p = ps.tile([128, BC], f32,
                                    name=f"pc_t{t}b{bc}h{ht}", tag="ps")
                        n = KD + (KHP if t > 0 else 0)
                        i = 0
                        for kd in range(KD):
                            wt = wja if kd < 8 else wjb
                            nc.tensor.matmul(
                                p, wt[:, kd % 8, :], xts[(bc, kd)],
                                start=(i == 0), stop=(i == n - 1))
                            i += 1
                        if t > 0:
                            for khp in range(KHP):
                                nc.tensor.matmul(
                                    p, u8c[khp][:, :, sl(ht)],
                                    rh8[(khp, bc)],
                                    start=False, stop=(i == n - 1),
                                    perf_mode=DR)
                                i += 1
                        hc = sb.tile([128, BC], bf16,
                                     name=f"hc_t{t}b{bc}h{ht}",
                                     tag="hc", bufs=HC_BUFS)
                        nc.scalar.activation(
                            hc, p, Act.Relu,
                            bias=bias_sb["c"][:, ht:ht + 1], scale=1.0 / SW)
                        hn = sb.tile([128, BC], bf16,
                                     name=f"h_t{t}b{bc}h{ht}",
                                     tag="h", bufs=H_BUFS)
                        if t == 0:
                            # h1 = (1-z)*hc = wbar*hc, on GPSIMD: the DVE
                            # FIFO is full of one-time weight casts at t0
                            nc.gpsimd.tensor_mul(hn, wbar[(ht, bc)], hc)
                        else:
                            # h' = h - wbar*(h - hc)
                            d_ = sb.tile([128, BC], f32,
                                         name=f"d_t{t}b{bc}h{ht}",
                                         tag="tmp1", bufs=DE_BUFS)
                            nc.vector.tensor_sub(d_, h[(ht, bc)], hc)
                            e_ = sb.tile([128, BC], f32,
                                         name=f"e_t{t}b{bc}h{ht}",
                                         tag="tmp2", bufs=DE_BUFS)
                            nc.vector.tensor_mul(e_, wbar[(ht, bc)], d_)
                            nc.vector.tensor_sub(hn, h[(ht, bc)], e_)
                        h_new[(ht, bc)] = hn
                        if t < T - 1:
                            # h8' = cast(h' * 8) into pair slot
                            khp_i, i_h = ht // 2, ht % 2
                            if i_h == 0:
                                h8_new[(khp_i, bc)] = sb.tile(
                                    [128, 2, BC], fp8,
                                    name=f"h8_t{t}b{bc}p{khp_i}",
                                    tag="h8", bufs=H8_BUFS)
                            nc.scalar.activation(
                                h8_new[(khp_i, bc)][:, i_h, :], hn,
                                Act.Copy, scale=SH)
                h = h_new
                h8 = h8_new

                # tail: next step's r weights first, then bc1 transposes +
                # casts (bc0 was emitted inside the c loop)
                if t < T - 1:
                    w8r_pre = load_w8("r", t + 1)
                    u8r_pre = load_u8("r", t + 1)
                    for half in range(2):
                        emit_xbars_quad(t + 1, 1, half)
                    emit_xt8_bc(t + 1, 1)

            # ---- final projection: y = relu(hT.T @ w_out + b_out) ----
            for half in range(2):
                wo = {}
                for kh in range(KH):
                    s32 = sb.tile([128, 1024], f32, name=f"wo32_{kh}_{half}",
                                  tag="s32", bufs=S32_BUFS)
                    nc.sync.dma_start(s32, wout_in[sl(kh), sl(half, 1024)])
                    wt = sb.tile([128, 1024], bf16, name=f"wo_{kh}_{half}",
                                 tag="w8", bufs=W8_BUFS)
                    nc.vector.tensor_copy(wt, s32)
                    wo[kh] = wt
                for uc in (2 * half, 2 * half + 1):
                    for bc in range(NBC):
                        for bi in range(NBI):
                            p = ps.tile([128, BC], f32,
                                        name=f"po_b{bc}i{bi}u{uc}", tag="ps")
                            for kh in range(KH):
                                nc.tensor.matmul(
                                    p, h[(kh, bc)][:, sl(bi)],
                                    wo[kh][:, sl(uc % 2, 512)],
                                    start=(kh == 0), stop=(kh == KH - 1))
                            ot = sb.tile([128, BC], f32,
                                         name=f"ot_b{bc}i{bi}u{uc}",
                                         tag="otmp", bufs=2)
                            nc.vector.tensor_add(ot, p,
                                                 bout_sb[:, sl(uc, BC)])
                            oo = sb.tile([128, BC], f32,
                                         name=f"oo_b{bc}i{bi}u{uc}",
                                         tag="o", bufs=2)
                            nc.scalar.activation(oo, ot, Act.Relu)
                            nc.sync.dma_start(
                                y_out[bc * BC + bi * 128:
                                      bc * BC + (bi + 1) * 128,
                                      sl(uc, BC)], oo)

    nc.finalize()
    return nc


_nc_cache = None


def _get_nc():
    global _nc_cache
    if _nc_cache is None:
        _nc_cache = _build()
    return _nc_cache


def run(inputs, trace=False):
    """Run on 8 cores; returns (y_full, BassKernelResults)."""
    from concourse.bass_utils import run_bass_kernel_spmd

    nc = _get_nc()
    arrs = {k: np.ascontiguousarray(np.asarray(v, dtype=np.float32))
            for k, v in inputs.items()}
    in_maps = []
    for c in range(N_CORES):
        m = {k: v for k, v in arrs.items() if k != "x"}
        m["x"] = np.ascontiguousarray(arrs["x"][c * B_LOC:(c + 1) * B_LOC])
        in_maps.append(m)
    res = run_bass_kernel_spmd(nc, in_maps, core_ids=list(range(N_CORES)),
                               trace=trace)
    y = np.concatenate([res.results[c]["y"] for c in range(N_CORES)], axis=0)
    return y.astype(np.float32), res


def kernel(**inputs) -> np.ndarray:
    y, _ = run(inputs, trace=False)
    return y


# revision 52
# speedup vs baseline: 1.0399x; 1.0399x over previous
"""Trainium2 Bass kernel for a GRU-like recurrent cell (4 unrolled timesteps)
with relu candidate and final output projection.

Math (per batch row, h0 = 0):
  for t in 0..3:
    r = sigmoid(x_t @ wr + h @ Ur + br)        # skipped at t=0 (r*h = 0)
    z = sigmoid(x_t @ wz + h @ Uz + bz)
    c = relu  (x_t @ wh + (r*h) @ Uh + bh)
    h = (1-z)*c + z*h
  y = relu(h @ w_out + b_out)

Distribution: data-parallel over batch across 8 cores (x/y sharded on dim 0,
weights replicated). Each core computes B_LOC=1024 rows.

fp8 (e4m3) DoubleRow matmuls (2 contraction elems/partition/cycle) for the
error-tolerant sites (x@wr, x@wz, h@Ur, h@Uz, (r*h)@Uh); bf16 for the
error-critical sites (x@wh, h@w_out).  Scales: weights x256, U matrices x32,
h-state x8 -> every gate PSUM holds 256*(true preactivation), dequantized for
free by ACT (out = func(psum*(1/256) + bias)).  wh is scaled x256 in bf16 so
the mixed bf16+fp8 accumulation shares one PSUM scale.

z is stored as wbar = 1-z = sigmoid(-pre) in bf16: saturated gates (z ~ 1,
driven by the positive-mean h @ Uz sum) need relative precision on 1-z.
h update: h' = h - wbar*(h - hc); t=0: h1 = wbar*hc (on the otherwise-idle
GPSIMD engine, so the one-time DVE weight-cast chain cannot stall it).

All recurrent state is kept TRANSPOSED in SBUF as [h_partition, batch_free]
tiles.  x is cast fp32->bf16 into a DRAM scratch (ACT) then loaded transposed
via the 2-byte xbar DMA transpose (sync queue only - xbar descriptors are
broken on the ACT HWDGE queue); fp8 copies of the transposed tiles are made
by ACT casts in SBUF.

Weights are loaded fp32 once and cast+scaled on-chip (DVE): wr/wz into packed
fp8 pair-tile DRAM staging ([128, 2, 1024] fp8: sub-tile i = contraction rows
128i..128i+127 of a 256-row pair block), U matrices likewise ([128, 2, 1024]
with 128-row sub-blocks), wh into per-output-block packed bf16 staging
([ht][kd][128][128]) so the candidate stage keeps only 3 of 8 column windows
in SBUF.  Everything is re-streamed per step from staging on the sync queue
in consumption order; all one-time fp32 loads ride the sync queue in deadline
order (Q7/SWDGE measured ~44 GB/s - too slow for anything deadline-bound).
"""
import numpy as np

B_FULL, T, D, H, U = 8192, 4, 2048, 1024, 2048
N_CORES = 8
B_LOC = B_FULL // N_CORES   # 1024
BC = 512                    # batch columns per moving-operand chunk
NBC = B_LOC // BC           # 2
KD = D // 128               # 16 contraction tiles for x @ W
KDP = KD // 2               # 8 fp8 pair tiles
KH = H // 128               # 8 contraction tiles for h @ U
KHP = KH // 2               # 4 fp8 pair tiles
NUC = U // BC               # 4 output column chunks
NBI = BC // 128             # 4 output row tiles per chunk

SW = 256.0                  # weight scale (wr, wz, wh)
SU = 32.0                   # U matrix scale
SH = 8.0                    # h state scale  (SW = SU * SH)

S32_BUFS = 2       # one-time weight-cast staging (own tag: its slot chain
                   # drains at prologue-DMA pace and must not gate the pipe)
XS32_BUFS = 3      # x-pipe fp32 staging (dedicated tag)
XS16_BUFS = 2
XT16_BUFS = 45     # 32 hard-live in c stage + 13 early next-step transposes
XT8_BUFS = 16      # 16 hard-live per step (8 kdp x 2 bc)
W8_BUFS = 14
WHP_BUFS = 5
WH16N_BUFS = 2
H_BUFS = 17
H8_BUFS = 8
RH8_BUFS = 8       # all 8 (4 khp x 2 bc) live through c stage
WBAR_BUFS = 16     # all 16 (8 ht x 2 bc) live into c stage
R_BUFS = 3
HC_BUFS = 2
DE_BUFS = 2


def _build():
    import concourse.mybir as mybir
    import concourse.tile as tile
    import concourse.bass as bass
    from concourse import bacc

    f32 = mybir.dt.float32
    bf16 = mybir.dt.bfloat16
    fp8 = mybir.dt.float8e4
    Act = mybir.ActivationFunctionType
    DR = mybir.MatmulPerfMode.DoubleRow

    def sl(i, step=128):
        return slice(i * step, (i + 1) * step)

    nc = bacc.Bacc("TRN2", target_bir_lowering=False, name="gru_fp8")

    x_in = nc.dram_tensor("x", [B_LOC, T, D], f32, kind="ExternalInput")
    w_in = {
        "r": nc.dram_tensor("wr", [D, H], f32, kind="ExternalInput"),
        "z": nc.dram_tensor("wz", [D, H], f32, kind="ExternalInput"),
        "c": nc.dram_tensor("wh", [D, H], f32, kind="ExternalInput"),
    }
    u_in = {
        "r": nc.dram_tensor("Ur", [H, H], f32, kind="ExternalInput"),
        "z": nc.dram_tensor("Uz", [H, H], f32, kind="ExternalInput"),
        "c": nc.dram_tensor("Uh", [H, H], f32, kind="ExternalInput"),
    }
    b_in = {
        "r": nc.dram_tensor("br", [H], f32, kind="ExternalInput"),
        "z": nc.dram_tensor("bz", [H], f32, kind="ExternalInput"),
        "c": nc.dram_tensor("bh", [H], f32, kind="ExternalInput"),
    }
    wout_in = nc.dram_tensor("w_out", [H, U], f32, kind="ExternalInput")
    bout_in = nc.dram_tensor("b_out", [U], f32, kind="ExternalInput")
    y_out = nc.dram_tensor("y", [B_LOC, U], f32, kind="ExternalOutput")
    xbf = nc.dram_tensor("xbf", [T, B_LOC, D], bf16)
    # packed fp8 pair-tile staging: [kdp][part 128][sub 2][col 1024]
    w8s = {g: nc.dram_tensor(f"w8s_{g}", [KDP, 128, 2, H], fp8)
           for g in ("r", "z")}
    u8s = {g: nc.dram_tensor(f"u8s_{g}", [KHP, 128, 2, H], fp8)
           for g in ("r", "z", "c")}
    # wh packed per ht: [ht 8][kd 16][part 128][col 128] bf16 (x256)
    whp_s = nc.dram_tensor("whp_s", [KH, KD, 128, 128], bf16)

    with tile.TileContext(nc) as tc:
        with tc.tile_pool(name="sb", bufs=1) as sb, \
             tc.tile_pool(name="ps", bufs=6, space="PSUM") as ps:

            # ---- biases: [128, KH] per-partition scalars per h-tile ----
            bias_sb = {}
            for g in ("r", "z", "c"):
                bt = sb.tile([128, KH], f32, name=f"bias_{g}", tag=f"bias_{g}")
                nc.sync.dma_start(bt, b_in[g].ap().rearrange("(kh p) -> p kh", p=128))
                bias_sb[g] = bt
            # negated bz for wbar = sigmoid(-pre - bz)
            bzn_sb = sb.tile([128, KH], f32, name="bzn", tag="bzn")
            nc.vector.tensor_scalar_mul(bzn_sb, bias_sb["z"], -1.0)
            # output bias broadcast across partitions: [128, U]
            bout_ap = bout_in.ap()
            bout_bcast_src = bass.AP(
                tensor=bout_ap.tensor, offset=bout_ap.offset,
                ap=[[0, 128]] + list(bout_ap.ap))
            bout_sb = sb.tile([128, U], bf16, name="bout_sb", tag="bout_sb")
            nc.gpsimd.dma_start(bout_sb, bout_bcast_src)

            # ---- x pipeline: fp32 -> bf16 xbf scratch, then xbar loads ----
            xts_all = {}   # (t) -> {(bc, kd): bf16 [128, 512] tile}
            xt8_all = {}   # (t) -> {(bc, kdp): fp8 [128, 2, 512] tile}

            def emit_x_pipe_quad(t, bc, half):
                for blk in range(4):
                    b0 = bc * BC + blk * 128
                    xs32 = sb.tile([128, 1024], f32,
                                   name=f"xs32_t{t}b{bc}h{half}k{blk}",
                                   tag="xs32", bufs=XS32_BUFS)
                    nc.sync.dma_start(
                        xs32, x_in[b0:b0 + 128, t, sl(half, 1024)])
                    xs16 = sb.tile([128, 1024], bf16,
                                   name=f"xs16_t{t}b{bc}h{half}k{blk}",
                                   tag="xs16", bufs=XS16_BUFS)
                    nc.scalar.copy(xs16, xs32)
                    nc.sync.dma_start(
                        xbf[t, b0:b0 + 128, sl(half, 1024)], xs16)

            def emit_xbars_kds(t, bc, kds):
                xts = xts_all.setdefault(t, {})
                for kd in kds:
                    xt_t = sb.tile([128, BC], bf16,
                                   name=f"xt_t{t}b{bc}k{kd}", tag="xt",
                                   bufs=XT16_BUFS)
                    nc.sync.dma_start(
                        xt_t, xbf[t, sl(bc, BC), sl(kd)], transpose=True)
                    xts[(bc, kd)] = xt_t

            def emit_xbars_quad(t, bc, half):
                emit_xbars_kds(t, bc, range(half * 8, half * 8 + 8))

            def emit_xt8_bc(t, bc, kdps=None):
                # ACT casts bf16 -> fp8 pair tiles [128, 2, 512]
                xts = xts_all[t]
                x8 = xt8_all.setdefault(t, {})
                for kdp in kdps if kdps is not None else range(KDP):
                    t8 = sb.tile([128, 2, BC], fp8,
                                 name=f"xt8_t{t}b{bc}p{kdp}", tag="xt8",
                                 bufs=XT8_BUFS)
                    for i in range(2):
                        nc.scalar.copy(t8[:, i, :], xts[(bc, 2 * kdp + i)])
                    x8[(bc, kdp)] = t8

            def emit_xt8(t):
                for bc in range(NBC):
                    emit_xt8_bc(t, bc)

            # ---- one-time weight cast + pack-store pipelines (all sync) ----
            def emit_w8_cast(g, pair_range=None, retain=False):
                # w[g] fp32 [D, H] -> pair tiles [128, 2, H] fp8 x SW
                tiles = []
                for kdp in pair_range if pair_range is not None else range(KDP):
                    w8t = sb.tile([128, 2, H], fp8, name=f"w8c_{g}{kdp}",
                                  tag="w8", bufs=W8_BUFS)
                    for i in range(2):
                        s32 = sb.tile([128, 1024], f32,
                                      name=f"wc32_{g}{kdp}i{i}",
                                      tag="s32", bufs=S32_BUFS)
                        nc.sync.dma_start(s32, w_in[g][sl(2 * kdp + i), :])
                        nc.vector.tensor_scalar_mul(w8t[:, i, :], s32, SW)
                    nc.sync.dma_start(w8s[g][kdp], w8t)
                    if retain:
                        tiles.append(w8t)
                return tiles

            def emit_u8_cast(g, ld=None):
                ld = ld or nc.sync
                for khp in range(KHP):
                    u8t = sb.tile([128, 2, H], fp8, name=f"u8c_{g}{khp}",
                                  tag="w8", bufs=W8_BUFS)
                    for i in range(2):
                        s32 = sb.tile([128, 1024], f32,
                                      name=f"uc32_{g}{khp}i{i}",
                                      tag="s32", bufs=S32_BUFS)
                        ld.dma_start(s32, u_in[g][sl(2 * khp + i), :])
                        nc.vector.tensor_scalar_mul(u8t[:, i, :], s32, SU)
                    nc.sync.dma_start(u8s[g][khp], u8t)

            def emit_wh_cast():
                # wh fp32 [D, H] -> x256 bf16, packed [ht][kd][128][128]
                whp_ap = whp_s.ap()
                for kd in range(KD):
                    s32 = sb.tile([128, 1024], f32, name=f"whc32_{kd}",
                                  tag="s32", bufs=S32_BUFS)
                    nc.sync.dma_start(s32, w_in["c"][sl(kd), :])
                    w16 = sb.tile([128, KH, 128], bf16, name=f"wh16n_{kd}",
                                  tag="wh16n", bufs=WH16N_BUFS)
                    nc.vector.tensor_scalar_mul(w16, s32, SW)
                    # store (p, ht, m) -> whp_s[ht, kd, p, m]
                    dst = bass.AP(
                        tensor=whp_ap.tensor,
                        offset=whp_ap.offset + kd * 128 * 128,
                        ap=[[128, 128], [KD * 128 * 128, KH], [1, 128]])
                    nc.sync.dma_start(dst, w16)

            # ---- streamed loads (sync queue) ----
            def load_w8(g, t):
                tiles = []
                for kdp in range(KDP):
                    w8t = sb.tile([128, 2, H], fp8, name=f"w8_{g}{kdp}_t{t}",
                                  tag="w8", bufs=W8_BUFS)
                    nc.sync.dma_start(w8t, w8s[g][kdp])
                    tiles.append(w8t)
                return tiles

            def load_u8(g, t):
                tiles = []
                for khp in range(KHP):
                    u8t = sb.tile([128, 2, H], fp8, name=f"u8_{g}{khp}_t{t}",
                                  tag="w8", bufs=W8_BUFS)
                    nc.sync.dma_start(u8t, u8s[g][khp])
                    tiles.append(u8t)
                return tiles

            def load_whp(ht, t):
                # 2 tiles of [128, 8, 128] covering kd 0-7 / 8-15 for one ht
                whp_ap = whp_s.ap()
                tiles = []
                for j in range(2):
                    wt = sb.tile([128, 8, 128], bf16,
                                 name=f"whp_t{t}h{ht}j{j}", tag="whp",
                                 bufs=WHP_BUFS)
                    src = bass.AP(
                        tensor=whp_ap.tensor,
                        offset=whp_ap.offset
                        + ht * KD * 128 * 128 + j * 8 * 128 * 128,
                        ap=[[128, 128], [128 * 128, 8], [1, 128]])
                    nc.sync.dma_start(wt, src)
                    tiles.append(wt)
                return tiles

            # ---- recurrent state ----
            h = {}      # (kh, bc) -> bf16 [128, BC] hidden state (transposed)
            h8 = {}     # (khp, bc) -> fp8 [128, 2, BC], value = 8*h

            # t0 prologue on sync, deadline-ordered: x pipe + xbars + wz
            # interleaved (all gate the first MMs), then wh (t0-c via the
            # whp round trip), whp windows 0-2, then wr/Ur (t1-r), Uz/Uh.
            wz8_t0 = []
            for bc in range(NBC):
                for half in range(2):
                    emit_x_pipe_quad(0, bc, half)
                    emit_xbars_quad(0, bc, half)
                    i2 = 2 * (2 * bc + half)
                    wz8_t0 += emit_w8_cast("z", pair_range=range(i2, i2 + 2),
                                           retain=True)
            emit_xt8(0)
            emit_wh_cast()
            whp_t0 = {k: load_whp(k, 0) for k in range(3)}
            emit_w8_cast("r")
            emit_u8_cast("r")
            # Uz/Uh fp32 ride the ACT HWDGE queue (plain loads only - xbar
            # transposes are broken there), issued after the xt8(0) casts
            emit_u8_cast("z", ld=nc.scalar)
            emit_u8_cast("c", ld=nc.scalar)

            for t in range(T):
                xts = xts_all[t]
                x8 = xt8_all[t]
                rh8 = {}
                wbar = {}

                # ---------- r stage (t >= 1) ----------
                if t > 0:
                    w8r, u8r = w8r_pre, u8r_pre   # loaded at prior step tail
                    for bc in range(NBC):
                        for ht in range(KH):
                            p = ps.tile([128, BC], f32,
                                        name=f"pr_t{t}b{bc}h{ht}", tag="ps")
                            n = KDP + KHP
                            i = 0
                            # h@U first: x-independent work covers the
                            # step-boundary xbar/cast latency
                            for khp in range(KHP):
                                nc.tensor.matmul(
                                    p, u8r[khp][:, :, sl(ht)], h8[(khp, bc)],
                                    start=(i == 0), stop=False,
                                    perf_mode=DR)
                                i += 1
                            for kdp in range(KDP):
                                nc.tensor.matmul(
                                    p, w8r[kdp][:, :, sl(ht)], x8[(bc, kdp)],
                                    start=False, stop=(i == n - 1),
                                    perf_mode=DR)
                                i += 1
                            rt = sb.tile([128, BC], f32,
                                         name=f"r_t{t}b{bc}h{ht}",
                                         tag="r", bufs=R_BUFS)
                            nc.scalar.activation(
                                rt, p, Act.Sigmoid,
                                bias=bias_sb["r"][:, ht:ht + 1],
                                scale=1.0 / SW)
                            # rh8 pair tile: alloc at even ht, fill halves
                            khp_i, i_h = ht // 2, ht % 2
                            if i_h == 0:
                                rh8[(khp_i, bc)] = sb.tile(
                                    [128, 2, BC], fp8,
                                    name=f"rh8_t{t}b{bc}p{khp_i}",
                                    tag="rh8", bufs=RH8_BUFS)
                            nc.vector.tensor_mul(
                                rh8[(khp_i, bc)][:, i_h, :], rt,
                                h8[(ht // 2, bc)][:, ht % 2, :])

                # ---------- z stage ----------
                if t == 0:
                    w8z, u8z = wz8_t0, None
                else:
                    w8z = load_w8("z", t)
                    u8z = load_u8("z", t)
                for bc in range(NBC):
                    for ht in range(KH):
                        p = ps.tile([128, BC], f32,
                                    name=f"pz_t{t}b{bc}h{ht}", tag="ps")
                        n = KDP + (KHP if t > 0 else 0)
                        i = 0
                        if t > 0:
                            for khp in range(KHP):
                                nc.tensor.matmul(
                                    p, u8z[khp][:, :, sl(ht)], h8[(khp, bc)],
                                    start=(i == 0), stop=False,
                                    perf_mode=DR)
                                i += 1
                        for kdp in range(KDP):
                            nc.tensor.matmul(
                                p, w8z[kdp][:, :, sl(ht)], x8[(bc, kdp)],
                                start=(i == 0), stop=(i == n - 1),
                                perf_mode=DR)
                            i += 1
                        wb = sb.tile([128, BC], bf16,
                                     name=f"wbar_t{t}b{bc}h{ht}",
                                     tag="wbar", bufs=WBAR_BUFS)
                        nc.scalar.activation(
                            wb, p, Act.Sigmoid,
                            bias=bzn_sb[:, ht:ht + 1], scale=-1.0 / SW)
                        wbar[(ht, bc)] = wb

                # ---------- c stage + h update (ht-outer for whp windows) ---
                u8c = load_u8("c", t) if t > 0 else None
                h_new = {}
                h8_new = {}
                if t == 0:
                    whp_tiles = whp_t0
                else:
                    whp_tiles = {0: load_whp(0, t), 1: load_whp(1, t),
                                 2: load_whp(2, t)}
                for ht in range(KH):
                    if ht + 3 < KH:
                        whp_tiles[ht + 3] = load_whp(ht + 3, t)
                    if t < T - 1:
                        # next step's x pipe + bc0 transposes interleaved
                        # under c compute (13 xt slots are free); each
                        # xbar trails its source quad by >= 2 iterations
                        if ht % 2 == 0:
                            emit_x_pipe_quad(t + 1, ht // 4, (ht // 2) % 2)
                        if 2 <= ht < 5:
                            emit_xbars_kds(t + 1, 0,
                                           range(4 * (ht - 2), 4 * ht - 4))
                        elif ht == 6:
                            emit_xt8_bc(t + 1, 0, kdps=range(0, 3))
                        elif ht == 7:
                            emit_xt8_bc(t + 1, 0, kdps=range(3, 6))
                    wja, wjb = whp_tiles[ht]
                    for bc in range(NBC):
                        p = ps.tile([128, BC], f32,
                                    name=f"pc_t{t}b{bc}h{ht}", tag="ps")
                        n = KD + (KHP if t > 0 else 0)
                        i = 0
                        for kd in range(KD):
                            wt = wja if kd < 8 else wjb
                            nc.tensor.matmul(
                                p, wt[:, kd % 8, :], xts[(bc, kd)],
                                start=(i == 0), stop=(i == n - 1))
                            i += 1
                        if t > 0:
                            for khp in range(KHP):
                                nc.tensor.matmul(
                                    p, u8c[khp][:, :, sl(ht)],
                                    rh8[(khp, bc)],
                                    start=False, stop=(i == n - 1),
                                    perf_mode=DR)
                                i += 1
                        hc = sb.tile([128, BC], bf16,
                                     name=f"hc_t{t}b{bc}h{ht}",
                                     tag="hc", bufs=HC_BUFS)
                        nc.scalar.activation(
                            hc, p, Act.Relu,
                            bias=bias_sb["c"][:, ht:ht + 1], scale=1.0 / SW)
                        hn = sb.tile([128, BC], bf16,
                                     name=f"h_t{t}b{bc}h{ht}",
                                     tag="h", bufs=H_BUFS)
                        if t == 0:
                            # h1 = (1-z)*hc = wbar*hc, on GPSIMD: the DVE
                            # FIFO is full of one-time weight casts at t0
                            nc.gpsimd.tensor_mul(hn, wbar[(ht, bc)], hc)
                        else:
                            # h' = h - wbar*(h - hc)
                            d_ = sb.tile([128, BC], f32,
                                         name=f"d_t{t}b{bc}h{ht}",
                                         tag="tmp1", bufs=DE_BUFS)
                            nc.vector.tensor_sub(d_, h[(ht, bc)], hc)
                            e_ = sb.tile([128, BC], f32,
                                         name=f"e_t{t}b{bc}h{ht}",
                                         tag="tmp2", bufs=DE_BUFS)
                            nc.vector.tensor_mul(e_, wbar[(ht, bc)], d_)
                            nc.vector.tensor_sub(hn, h[(ht, bc)], e_)
                        h_new[(ht, bc)] = hn
                        if t < T - 1:
                            # h8' = cast(h' * 8) into pair slot
                            khp_i, i_h = ht // 2, ht % 2
                            if i_h == 0:
                                h8_new[(khp_i, bc)] = sb.tile(
                                    [128, 2, BC], fp8,
                                    name=f"h8_t{t}b{bc}p{khp_i}",
                                    tag="h8", bufs=H8_BUFS)
                            nc.scalar.activation(
                                h8_new[(khp_i, bc)][:, i_h, :], hn,
                                Act.Copy, scale=SH)
                h = h_new
                h8 = h8_new

                # tail: next step's r weights first, then remaining
                # transposes + casts (bc0 kd0-11 ran inside the c loop)
                if t < T - 1:
                    w8r_pre = load_w8("r", t + 1)
                    u8r_pre = load_u8("r", t + 1)
                    emit_xbars_kds(t + 1, 0, range(12, 16))
                    emit_xt8_bc(t + 1, 0, kdps=range(6, 8))
                    for half in range(2):
                        emit_xbars_quad(t + 1, 1, half)
                    emit_xt8_bc(t + 1, 1)

            # ---- final projection: y = relu(hT.T @ w_out + b_out) ----
            for half in range(2):
                wo = {}
                for kh in range(KH):
                    s32 = sb.tile([128, 1024], f32, name=f"wo32_{kh}_{half}",
                                  tag="s32", bufs=S32_BUFS)
                    nc.sync.dma_start(s32, wout_in[sl(kh), sl(half, 1024)])
                    wt = sb.tile([128, 1024], bf16, name=f"wo_{kh}_{half}",
                                 tag="w8", bufs=W8_BUFS)
                    nc.vector.tensor_copy(wt, s32)
                    wo[kh] = wt
                for uc in (2 * half, 2 * half + 1):
                    for bc in range(NBC):
                        for bi in range(NBI):
                            p = ps.tile([128, BC], f32,
                                        name=f"po_b{bc}i{bi}u{uc}", tag="ps")
                            for kh in range(KH):
                                nc.tensor.matmul(
                                    p, h[(kh, bc)][:, sl(bi)],
                                    wo[kh][:, sl(uc % 2, 512)],
                                    start=(kh == 0), stop=(kh == KH - 1))
                            ot = sb.tile([128, BC], f32,
                                         name=f"ot_b{bc}i{bi}u{uc}",
                                         tag="otmp", bufs=2)
                            nc.vector.tensor_add(ot, p,
                                                 bout_sb[:, sl(uc, BC)])
                            oo = sb.tile([128, BC], f32,
                                         name=f"oo_b{bc}i{bi}u{uc}",
                                         tag="o", bufs=2)
                            nc.scalar.activation(oo, ot, Act.Relu)
                            nc.sync.dma_start(
                                y_out[bc * BC + bi * 128:
                                      bc * BC + (bi + 1) * 128,
                                      sl(uc, BC)], oo)

    nc.finalize()
    return nc


_nc_cache = None


def _get_nc():
    global _nc_cache
    if _nc_cache is None:
        _nc_cache = _build()
    return _nc_cache


def run(inputs, trace=False):
    """Run on 8 cores; returns (y_full, BassKernelResults)."""
    from concourse.bass_utils import run_bass_kernel_spmd

    nc = _get_nc()
    arrs = {k: np.ascontiguousarray(np.asarray(v, dtype=np.float32))
            for k, v in inputs.items()}
    in_maps = []
    for c in range(N_CORES):
        m = {k: v for k, v in arrs.items() if k != "x"}
        m["x"] = np.ascontiguousarray(arrs["x"][c * B_LOC:(c + 1) * B_LOC])
        in_maps.append(m)
    res = run_bass_kernel_spmd(nc, in_maps, core_ids=list(range(N_CORES)),
                               trace=trace)
    y = np.concatenate([res.results[c]["y"] for c in range(N_CORES)], axis=0)
    return y.astype(np.float32), res


def kernel(**inputs) -> np.ndarray:
    y, _ = run(inputs, trace=False)
    return y


# revision 56
# speedup vs baseline: 1.1000x; 1.0578x over previous
"""Trainium2 Bass kernel for a GRU-like recurrent cell (4 unrolled timesteps)
with relu candidate and final output projection.

Math (per batch row, h0 = 0):
  for t in 0..3:
    r = sigmoid(x_t @ wr + h @ Ur + br)        # skipped at t=0 (r*h = 0)
    z = sigmoid(x_t @ wz + h @ Uz + bz)
    c = relu  (x_t @ wh + (r*h) @ Uh + bh)
    h = (1-z)*c + z*h
  y = relu(h @ w_out + b_out)

Distribution: data-parallel over batch across 8 cores (x/y sharded on dim 0,
weights replicated). Each core computes B_LOC=1024 rows.

fp8 (e4m3) DoubleRow matmuls (2 contraction elems/partition/cycle) for the
error-tolerant sites (x@wr, x@wz, h@Ur, h@Uz, (r*h)@Uh); bf16 for the
error-critical sites (x@wh, h@w_out).  Scales: weights x256, U matrices x32,
h-state x8 -> every gate PSUM holds 256*(true preactivation), dequantized for
free by ACT (out = func(psum*(1/256) + bias)).  wh is scaled x256 in bf16 so
the mixed bf16+fp8 accumulation shares one PSUM scale.

z is stored as wbar = 1-z = sigmoid(-pre) in bf16: saturated gates (z ~ 1,
driven by the positive-mean h @ Uz sum) need relative precision on 1-z.
h update: h' = h - wbar*(h - hc); t=0: h1 = wbar*hc (on the otherwise-idle
GPSIMD engine, so the one-time DVE weight-cast chain cannot stall it).

All recurrent state is kept TRANSPOSED in SBUF as [h_partition, batch_free]
tiles.  x is cast fp32->bf16 into a DRAM scratch (ACT) then loaded transposed
via the 2-byte xbar DMA transpose (sync queue only - xbar descriptors are
broken on the ACT HWDGE queue); fp8 copies of the transposed tiles are made
by ACT casts in SBUF.

Weights are loaded fp32 once and cast+scaled on-chip (DVE): wr/wz into packed
fp8 pair-tile DRAM staging ([128, 2, 1024] fp8: sub-tile i = contraction rows
128i..128i+127 of a 256-row pair block), U matrices likewise ([128, 2, 1024]
with 128-row sub-blocks), wh into per-output-block packed bf16 staging
([ht][kd][128][128]) so the candidate stage keeps only 3 of 8 column windows
in SBUF.  Everything is re-streamed per step from staging on the sync queue
in consumption order; all one-time fp32 loads ride the sync queue in deadline
order (Q7/SWDGE measured ~44 GB/s - too slow for anything deadline-bound).
"""
import numpy as np

B_FULL, T, D, H, U = 8192, 4, 2048, 1024, 2048
N_CORES = 8
B_LOC = B_FULL // N_CORES   # 1024
BC = 512                    # batch columns per moving-operand chunk
NBC = B_LOC // BC           # 2
KD = D // 128               # 16 contraction tiles for x @ W
KDP = KD // 2               # 8 fp8 pair tiles
KH = H // 128               # 8 contraction tiles for h @ U
KHP = KH // 2               # 4 fp8 pair tiles
NUC = U // BC               # 4 output column chunks
NBI = BC // 128             # 4 output row tiles per chunk

SW = 256.0                  # weight scale (wr, wz, wh)
SU = 32.0                   # U matrix scale
SH = 8.0                    # h state scale  (SW = SU * SH)

S32_BUFS = 2       # one-time weight-cast staging (own tag: its slot chain
                   # drains at prologue-DMA pace and must not gate the pipe)
XS32_BUFS = 3      # x-pipe fp32 staging (dedicated tag)
XS16_BUFS = 2
XT16_BUFS = 33     # 32 hard-live in c stage + 1
XT8_BUFS = 16      # 16 hard-live per step (8 kdp x 2 bc)
W8_BUFS = 17
WHP_BUFS = 5
WH16N_BUFS = 2
H_BUFS = 18
H8_BUFS = 9
RH8_BUFS = 8       # all 8 (4 khp x 2 bc) live through c stage
WBAR_BUFS = 17     # all 16 (8 ht x 2 bc) live into c stage + 1
R_BUFS = 3
HC_BUFS = 3
DE_BUFS = 3


def _build():
    import concourse.mybir as mybir
    import concourse.tile as tile
    import concourse.bass as bass
    from concourse import bacc

    f32 = mybir.dt.float32
    bf16 = mybir.dt.bfloat16
    fp8 = mybir.dt.float8e4
    Act = mybir.ActivationFunctionType
    DR = mybir.MatmulPerfMode.DoubleRow

    def sl(i, step=128):
        return slice(i * step, (i + 1) * step)

    nc = bacc.Bacc("TRN2", target_bir_lowering=False, name="gru_fp8")

    x_in = nc.dram_tensor("x", [B_LOC, T, D], f32, kind="ExternalInput")
    w_in = {
        "r": nc.dram_tensor("wr", [D, H], f32, kind="ExternalInput"),
        "z": nc.dram_tensor("wz", [D, H], f32, kind="ExternalInput"),
        "c": nc.dram_tensor("wh", [D, H], f32, kind="ExternalInput"),
    }
    u_in = {
        "r": nc.dram_tensor("Ur", [H, H], f32, kind="ExternalInput"),
        "z": nc.dram_tensor("Uz", [H, H], f32, kind="ExternalInput"),
        "c": nc.dram_tensor("Uh", [H, H], f32, kind="ExternalInput"),
    }
    b_in = {
        "r": nc.dram_tensor("br", [H], f32, kind="ExternalInput"),
        "z": nc.dram_tensor("bz", [H], f32, kind="ExternalInput"),
        "c": nc.dram_tensor("bh", [H], f32, kind="ExternalInput"),
    }
    wout_in = nc.dram_tensor("w_out", [H, U], f32, kind="ExternalInput")
    bout_in = nc.dram_tensor("b_out", [U], f32, kind="ExternalInput")
    y_out = nc.dram_tensor("y", [B_LOC, U], f32, kind="ExternalOutput")
    xbf = nc.dram_tensor("xbf", [T, B_LOC, D], bf16)
    # packed fp8 pair-tile staging: [kdp][part 128][sub 2][col 1024]
    w8s = {g: nc.dram_tensor(f"w8s_{g}", [KDP, 128, 2, H], fp8)
           for g in ("r", "z")}
    u8s = {g: nc.dram_tensor(f"u8s_{g}", [KHP, 128, 2, H], fp8)
           for g in ("r", "z", "c")}
    # wh packed per ht: [ht 8][kd 16][part 128][col 128] bf16 (x256)
    whp_s = nc.dram_tensor("whp_s", [KH, KD, 128, 128], bf16)

    with tile.TileContext(nc) as tc:
        with tc.tile_pool(name="sb", bufs=1) as sb, \
             tc.tile_pool(name="ps", bufs=6, space="PSUM") as ps:

            # ---- biases: [128, KH] per-partition scalars per h-tile ----
            bias_sb = {}
            for g in ("r", "z", "c"):
                bt = sb.tile([128, KH], f32, name=f"bias_{g}", tag=f"bias_{g}")
                nc.sync.dma_start(bt, b_in[g].ap().rearrange("(kh p) -> p kh", p=128))
                bias_sb[g] = bt
            # negated bz for wbar = sigmoid(-pre - bz)
            bzn_sb = sb.tile([128, KH], f32, name="bzn", tag="bzn")
            nc.vector.tensor_scalar_mul(bzn_sb, bias_sb["z"], -1.0)
            # output bias broadcast across partitions: [128, U]
            bout_ap = bout_in.ap()
            bout_bcast_src = bass.AP(
                tensor=bout_ap.tensor, offset=bout_ap.offset,
                ap=[[0, 128]] + list(bout_ap.ap))
            bout_sb = sb.tile([128, U], bf16, name="bout_sb", tag="bout_sb")
            nc.gpsimd.dma_start(bout_sb, bout_bcast_src)

            # ---- x pipeline: fp32 -> bf16 xbf scratch, then xbar loads ----
            xts_all = {}   # (t) -> {(bc, kd): bf16 [128, 512] tile}
            xt8_all = {}   # (t) -> {(bc, kdp): fp8 [128, 2, 512] tile}

            def emit_x_pipe_quad(t, bc, half):
                for blk in range(4):
                    b0 = bc * BC + blk * 128
                    xs32 = sb.tile([128, 1024], f32,
                                   name=f"xs32_t{t}b{bc}h{half}k{blk}",
                                   tag="xs32", bufs=XS32_BUFS)
                    nc.sync.dma_start(
                        xs32, x_in[b0:b0 + 128, t, sl(half, 1024)])
                    xs16 = sb.tile([128, 1024], bf16,
                                   name=f"xs16_t{t}b{bc}h{half}k{blk}",
                                   tag="xs16", bufs=XS16_BUFS)
                    nc.scalar.copy(xs16, xs32)
                    nc.sync.dma_start(
                        xbf[t, b0:b0 + 128, sl(half, 1024)], xs16)

            def emit_xbars_kds(t, bc, kds):
                xts = xts_all.setdefault(t, {})
                for kd in kds:
                    xt_t = sb.tile([128, BC], bf16,
                                   name=f"xt_t{t}b{bc}k{kd}", tag="xt",
                                   bufs=XT16_BUFS)
                    nc.sync.dma_start(
                        xt_t, xbf[t, sl(bc, BC), sl(kd)], transpose=True)
                    xts[(bc, kd)] = xt_t

            def emit_xbars_quad(t, bc, half):
                emit_xbars_kds(t, bc, range(half * 8, half * 8 + 8))

            def emit_xt8_bc(t, bc, kdps=None):
                # ACT casts bf16 -> fp8 pair tiles [128, 2, 512]
                xts = xts_all[t]
                x8 = xt8_all.setdefault(t, {})
                for kdp in kdps if kdps is not None else range(KDP):
                    t8 = sb.tile([128, 2, BC], fp8,
                                 name=f"xt8_t{t}b{bc}p{kdp}", tag="xt8",
                                 bufs=XT8_BUFS)
                    for i in range(2):
                        nc.scalar.copy(t8[:, i, :], xts[(bc, 2 * kdp + i)])
                    x8[(bc, kdp)] = t8

            def emit_xt8(t):
                for bc in range(NBC):
                    emit_xt8_bc(t, bc)

            # ---- one-time weight cast + pack-store pipelines (all sync) ----
            def emit_w8_cast(g, pair_range=None, retain=False):
                # w[g] fp32 [D, H] -> pair tiles [128, 2, H] fp8 x SW
                tiles = []
                for kdp in pair_range if pair_range is not None else range(KDP):
                    w8t = sb.tile([128, 2, H], fp8, name=f"w8c_{g}{kdp}",
                                  tag="w8", bufs=W8_BUFS)
                    for i in range(2):
                        s32 = sb.tile([128, 1024], f32,
                                      name=f"wc32_{g}{kdp}i{i}",
                                      tag="s32", bufs=S32_BUFS)
                        nc.sync.dma_start(s32, w_in[g][sl(2 * kdp + i), :])
                        nc.vector.tensor_scalar_mul(w8t[:, i, :], s32, SW)
                    nc.sync.dma_start(w8s[g][kdp], w8t)
                    if retain:
                        tiles.append(w8t)
                return tiles

            def emit_u8_cast(g, ld=None):
                ld = ld or nc.sync
                for khp in range(KHP):
                    u8t = sb.tile([128, 2, H], fp8, name=f"u8c_{g}{khp}",
                                  tag="w8", bufs=W8_BUFS)
                    for i in range(2):
                        s32 = sb.tile([128, 1024], f32,
                                      name=f"uc32_{g}{khp}i{i}",
                                      tag="s32", bufs=S32_BUFS)
                        ld.dma_start(s32, u_in[g][sl(2 * khp + i), :])
                        nc.vector.tensor_scalar_mul(u8t[:, i, :], s32, SU)
                    nc.sync.dma_start(u8s[g][khp], u8t)

            def emit_wh_cast():
                # wh fp32 [D, H] -> x256 bf16, packed [ht][kd][128][128]
                whp_ap = whp_s.ap()
                for kd in range(KD):
                    s32 = sb.tile([128, 1024], f32, name=f"whc32_{kd}",
                                  tag="s32", bufs=S32_BUFS)
                    nc.sync.dma_start(s32, w_in["c"][sl(kd), :])
                    w16 = sb.tile([128, KH, 128], bf16, name=f"wh16n_{kd}",
                                  tag="wh16n", bufs=WH16N_BUFS)
                    nc.vector.tensor_scalar_mul(w16, s32, SW)
                    # store (p, ht, m) -> whp_s[ht, kd, p, m]
                    dst = bass.AP(
                        tensor=whp_ap.tensor,
                        offset=whp_ap.offset + kd * 128 * 128,
                        ap=[[128, 128], [KD * 128 * 128, KH], [1, 128]])
                    nc.sync.dma_start(dst, w16)

            # ---- streamed loads (sync queue) ----
            def load_w8(g, t):
                tiles = []
                for kdp in range(KDP):
                    w8t = sb.tile([128, 2, H], fp8, name=f"w8_{g}{kdp}_t{t}",
                                  tag="w8", bufs=W8_BUFS)
                    nc.sync.dma_start(w8t, w8s[g][kdp])
                    tiles.append(w8t)
                return tiles

            def load_u8(g, t):
                tiles = []
                for khp in range(KHP):
                    u8t = sb.tile([128, 2, H], fp8, name=f"u8_{g}{khp}_t{t}",
                                  tag="w8", bufs=W8_BUFS)
                    nc.sync.dma_start(u8t, u8s[g][khp])
                    tiles.append(u8t)
                return tiles

            def load_whp(ht, t):
                # 2 tiles of [128, 8, 128] covering kd 0-7 / 8-15 for one ht
                whp_ap = whp_s.ap()
                tiles = []
                for j in range(2):
                    wt = sb.tile([128, 8, 128], bf16,
                                 name=f"whp_t{t}h{ht}j{j}", tag="whp",
                                 bufs=WHP_BUFS)
                    src = bass.AP(
                        tensor=whp_ap.tensor,
                        offset=whp_ap.offset
                        + ht * KD * 128 * 128 + j * 8 * 128 * 128,
                        ap=[[128, 128], [128 * 128, 8], [1, 128]])
                    nc.sync.dma_start(wt, src)
                    tiles.append(wt)
                return tiles

            # ---- recurrent state ----
            h = {}      # (kh, bc) -> bf16 [128, BC] hidden state (transposed)
            h8 = {}     # (khp, bc) -> fp8 [128, 2, BC], value = 8*h

            # t0 prologue on sync, deadline-ordered: x pipe + xbars + wz
            # interleaved (all gate the first MMs), then wh (t0-c via the
            # whp round trip), whp windows 0-2, then wr/Ur (t1-r), Uz/Uh.
            wz8_t0 = []
            for bc in range(NBC):
                for half in range(2):
                    emit_x_pipe_quad(0, bc, half)
                    emit_xbars_quad(0, bc, half)
                    i2 = 2 * (2 * bc + half)
                    wz8_t0 += emit_w8_cast("z", pair_range=range(i2, i2 + 2),
                                           retain=True)
            emit_xt8(0)
            emit_wh_cast()
            whp_t0 = {k: load_whp(k, 0) for k in range(3)}
            emit_w8_cast("r")
            emit_u8_cast("r")
            emit_u8_cast("z")
            emit_u8_cast("c")

            for t in range(T):
                xts = xts_all[t]
                x8 = xt8_all[t]
                rh8 = {}
                wbar = {}

                # ---------- r stage (t >= 1) ----------
                if t > 0:
                    w8r, u8r = w8r_pre, u8r_pre   # loaded at prior step tail
                    for bc in range(NBC):
                        for ht in range(KH):
                            p = ps.tile([128, BC], f32,
                                        name=f"pr_t{t}b{bc}h{ht}", tag="ps")
                            n = KDP + KHP
                            i = 0
                            # h@U first: x-independent work covers the
                            # step-boundary xbar/cast latency
                            for khp in range(KHP):
                                nc.tensor.matmul(
                                    p, u8r[khp][:, :, sl(ht)], h8[(khp, bc)],
                                    start=(i == 0), stop=False,
                                    perf_mode=DR)
                                i += 1
                            for kdp in range(KDP):
                                nc.tensor.matmul(
                                    p, w8r[kdp][:, :, sl(ht)], x8[(bc, kdp)],
                                    start=False, stop=(i == n - 1),
                                    perf_mode=DR)
                                i += 1
                            rt = sb.tile([128, BC], f32,
                                         name=f"r_t{t}b{bc}h{ht}",
                                         tag="r", bufs=R_BUFS)
                            nc.scalar.activation(
                                rt, p, Act.Sigmoid,
                                bias=bias_sb["r"][:, ht:ht + 1],
                                scale=1.0 / SW)
                            # rh8 pair tile: alloc at even ht, fill halves
                            khp_i, i_h = ht // 2, ht % 2
                            if i_h == 0:
                                rh8[(khp_i, bc)] = sb.tile(
                                    [128, 2, BC], fp8,
                                    name=f"rh8_t{t}b{bc}p{khp_i}",
                                    tag="rh8", bufs=RH8_BUFS)
                            nc.vector.tensor_mul(
                                rh8[(khp_i, bc)][:, i_h, :], rt,
                                h8[(ht // 2, bc)][:, ht % 2, :])

                # ---------- z stage ----------
                if t == 0:
                    w8z, u8z = wz8_t0, None
                else:
                    w8z = load_w8("z", t)
                    u8z = load_u8("z", t)
                for bc in range(NBC):
                    for ht in range(KH):
                        p = ps.tile([128, BC], f32,
                                    name=f"pz_t{t}b{bc}h{ht}", tag="ps")
                        n = KDP + (KHP if t > 0 else 0)
                        i = 0
                        if t > 0:
                            for khp in range(KHP):
                                nc.tensor.matmul(
                                    p, u8z[khp][:, :, sl(ht)], h8[(khp, bc)],
                                    start=(i == 0), stop=False,
                                    perf_mode=DR)
                                i += 1
                        for kdp in range(KDP):
                            nc.tensor.matmul(
                                p, w8z[kdp][:, :, sl(ht)], x8[(bc, kdp)],
                                start=(i == 0), stop=(i == n - 1),
                                perf_mode=DR)
                            i += 1
                        wb = sb.tile([128, BC], bf16,
                                     name=f"wbar_t{t}b{bc}h{ht}",
                                     tag="wbar", bufs=WBAR_BUFS)
                        nc.scalar.activation(
                            wb, p, Act.Sigmoid,
                            bias=bzn_sb[:, ht:ht + 1], scale=-1.0 / SW)
                        wbar[(ht, bc)] = wb

                # ---------- c stage + h update (ht-outer for whp windows) ---
                u8c = load_u8("c", t) if t > 0 else None
                h_new = {}
                h8_new = {}
                if t == 0:
                    whp_tiles = whp_t0
                else:
                    whp_tiles = {0: load_whp(0, t), 1: load_whp(1, t),
                                 2: load_whp(2, t)}
                for ht in range(KH):
                    if ht + 3 < KH:
                        whp_tiles[ht + 3] = load_whp(ht + 3, t)
                    if t < T - 1 and ht % 2 == 0:
                        # next step's x pipe interleaved under c compute
                        emit_x_pipe_quad(t + 1, ht // 4, (ht // 2) % 2)
                    wja, wjb = whp_tiles[ht]
                    for bc in range(NBC):
                        p = ps.tile([128, BC], f32,
                                    name=f"pc_t{t}b{bc}h{ht}", tag="ps")
                        n = KD + (KHP if t > 0 else 0)
                        i = 0
                        for kd in range(KD):
                            wt = wja if kd < 8 else wjb
                            nc.tensor.matmul(
                                p, wt[:, kd % 8, :], xts[(bc, kd)],
                                start=(i == 0), stop=(i == n - 1))
                            i += 1
                        if t > 0:
                            for khp in range(KHP):
                                nc.tensor.matmul(
                                    p, u8c[khp][:, :, sl(ht)],
                                    rh8[(khp, bc)],
                                    start=False, stop=(i == n - 1),
                                    perf_mode=DR)
                                i += 1
                        hc = sb.tile([128, BC], bf16,
                                     name=f"hc_t{t}b{bc}h{ht}",
                                     tag="hc", bufs=HC_BUFS)
                        nc.scalar.activation(
                            hc, p, Act.Relu,
                            bias=bias_sb["c"][:, ht:ht + 1], scale=1.0 / SW)
                        hn = sb.tile([128, BC], bf16,
                                     name=f"h_t{t}b{bc}h{ht}",
                                     tag="h", bufs=H_BUFS)
                        if t == 0:
                            # h1 = (1-z)*hc = wbar*hc, on GPSIMD: the DVE
                            # FIFO is full of one-time weight casts at t0
                            nc.gpsimd.tensor_mul(hn, wbar[(ht, bc)], hc)
                        else:
                            # h' = h - wbar*(h - hc)
                            d_ = sb.tile([128, BC], f32,
                                         name=f"d_t{t}b{bc}h{ht}",
                                         tag="tmp1", bufs=DE_BUFS)
                            nc.vector.tensor_sub(d_, h[(ht, bc)], hc)
                            e_ = sb.tile([128, BC], f32,
                                         name=f"e_t{t}b{bc}h{ht}",
                                         tag="tmp2", bufs=DE_BUFS)
                            nc.vector.tensor_mul(e_, wbar[(ht, bc)], d_)
                            nc.vector.tensor_sub(hn, h[(ht, bc)], e_)
                        h_new[(ht, bc)] = hn
                        if t < T - 1:
                            # h8' = cast(h' * 8) into pair slot
                            khp_i, i_h = ht // 2, ht % 2
                            if i_h == 0:
                                h8_new[(khp_i, bc)] = sb.tile(
                                    [128, 2, BC], fp8,
                                    name=f"h8_t{t}b{bc}p{khp_i}",
                                    tag="h8", bufs=H8_BUFS)
                            nc.scalar.activation(
                                h8_new[(khp_i, bc)][:, i_h, :], hn,
                                Act.Copy, scale=SH)
                h = h_new
                h8 = h8_new

                # tail: next step's r weights first, then transposes + casts
                if t < T - 1:
                    w8r_pre = load_w8("r", t + 1)
                    u8r_pre = load_u8("r", t + 1)
                    for bc in range(NBC):
                        for half in range(2):
                            emit_xbars_quad(t + 1, bc, half)
                    emit_xt8(t + 1)

            # ---- final projection: y = relu(hT.T @ w_out + b_out) ----
            for half in range(2):
                wo = {}
                for kh in range(KH):
                    s32 = sb.tile([128, 1024], f32, name=f"wo32_{kh}_{half}",
                                  tag="s32", bufs=S32_BUFS)
                    nc.sync.dma_start(s32, wout_in[sl(kh), sl(half, 1024)])
                    wt = sb.tile([128, 1024], bf16, name=f"wo_{kh}_{half}",
                                 tag="w8", bufs=W8_BUFS)
                    nc.vector.tensor_copy(wt, s32)
                    wo[kh] = wt
                for uc in (2 * half, 2 * half + 1):
                    for bc in range(NBC):
                        for bi in range(NBI):
                            p = ps.tile([128, BC], f32,
                                        name=f"po_b{bc}i{bi}u{uc}", tag="ps")
                            for kh in range(KH):
                                nc.tensor.matmul(
                                    p, h[(kh, bc)][:, sl(bi)],
                                    wo[kh][:, sl(uc % 2, 512)],
                                    start=(kh == 0), stop=(kh == KH - 1))
                            ot = sb.tile([128, BC], f32,
                                         name=f"ot_b{bc}i{bi}u{uc}",
                                         tag="otmp", bufs=2)
                            nc.vector.tensor_add(ot, p,
                                                 bout_sb[:, sl(uc, BC)])
                            oo = sb.tile([128, BC], f32,
                                         name=f"oo_b{bc}i{bi}u{uc}",
                                         tag="o", bufs=2)
                            nc.scalar.activation(oo, ot, Act.Relu)
                            nc.sync.dma_start(
                                y_out[bc * BC + bi * 128:
                                      bc * BC + (bi + 1) * 128,
                                      sl(uc, BC)], oo)

    nc.finalize()
    return nc


_nc_cache = None


def _get_nc():
    global _nc_cache
    if _nc_cache is None:
        _nc_cache = _build()
    return _nc_cache


def run(inputs, trace=False):
    """Run on 8 cores; returns (y_full, BassKernelResults)."""
    from concourse.bass_utils import run_bass_kernel_spmd

    nc = _get_nc()
    arrs = {k: np.ascontiguousarray(np.asarray(v, dtype=np.float32))
            for k, v in inputs.items()}
    in_maps = []
    for c in range(N_CORES):
        m = {k: v for k, v in arrs.items() if k != "x"}
        m["x"] = np.ascontiguousarray(arrs["x"][c * B_LOC:(c + 1) * B_LOC])
        in_maps.append(m)
    res = run_bass_kernel_spmd(nc, in_maps, core_ids=list(range(N_CORES)),
                               trace=trace)
    y = np.concatenate([res.results[c]["y"] for c in range(N_CORES)], axis=0)
    return y.astype(np.float32), res


def kernel(**inputs) -> np.ndarray:
    y, _ = run(inputs, trace=False)
    return y
